# revision 6
# baseline (speedup 1.0000x reference)
"""Trainium2 Bass kernel for the DWA middle layer (moe_routing).

Math (factored form of the reference):
    t     = h_A @ V_flat^T                      # [B, N*R]
    s     = t * repeat(alpha, R, axis=1)        # [B, N*R]
    h_T   = s @ U_flat^T + h_A @ W_base^T + [alpha, 1] @ [bias_pool; b_base]
    out   = LayerNorm(h_A + gamma * h_T) * ln_scale + ln_bias

Sharding: data-parallel over the batch dim (32 rows per core, 8 cores).
Weight matrices are replicated; on the host we only re-lay them out
(transpose/reshape/concat) so the contraction dim lands on SBUF
partitions — all arithmetic runs on device.

All PE matmuls keep the (small) activations stationary and stream the
weight matrices as the moving operand at N=512.
"""

import os
from contextlib import ExitStack

import numpy as np

import concourse.bacc as bacc
import concourse.mybir as mybir
import concourse.tile as tile
from concourse import bass_utils, masks

F32 = mybir.dt.float32
F32R = mybir.dt.float32r

D = 1024          # d_A == d_B
B_CORE = 32       # batch rows per core
N_EXP = 64        # experts
R_RANK = 16       # rank per expert
N_CORES = 8
KT = D // 128     # 8 contraction tiles of 128
NH = D // 512     # 2 moving halves of 512

# "f32r" = raw-fp32 single-pass PE mode (4x faster, slightly relaxed
# multiply precision); "f32" = full two-pass fp32.
MATMUL_MODE = os.environ.get("DWA_MATMUL_MODE", "f32r")
# Debug stage limiter: "loads" | "t" | "h" | "full"
STAGE = os.environ.get("DWA_STAGE", "full")

_COMPILED = {}


def _mm_ap(ap, mode):
    return ap.bitcast(F32R) if mode == "f32r" else ap


def _build(mode, stage="full"):
    nc = bacc.Bacc("TRN2", debug=False, num_devices=N_CORES)

    ha_d = nc.dram_tensor("ha", [B_CORE, D], F32, kind="ExternalInput")
    al_d = nc.dram_tensor("al", [B_CORE, N_EXP], F32, kind="ExternalInput")
    vt_d = nc.dram_tensor("vt", [D, D], F32, kind="ExternalInput")    # [a, nr]
    ut_d = nc.dram_tensor("ut", [D, D], F32, kind="ExternalInput")    # [nr, c]
    wt_d = nc.dram_tensor("wt", [D, D], F32, kind="ExternalInput")    # [a, c]
    bp_d = nc.dram_tensor("bp", [N_EXP + 1, D], F32, kind="ExternalInput")
    lns_d = nc.dram_tensor("lns", [1, D], F32, kind="ExternalInput")
    lnb_d = nc.dram_tensor("lnb", [1, D], F32, kind="ExternalInput")
    gm_d = nc.dram_tensor("gm", [1, 1], F32, kind="ExternalInput")
    out_d = nc.dram_tensor("out", [B_CORE, D], F32, kind="ExternalOutput")

    with ExitStack() as ctx:
        tc = ctx.enter_context(tile.TileContext(nc))
        _emit(ctx, tc, mode, stage, ha_d, al_d, vt_d, ut_d, wt_d, bp_d,
              lns_d, lnb_d, gm_d, out_d)

    nc.compile()
    return nc


def _emit(ctx, tc, mode, stage, ha_d, al_d, vt_d, ut_d, wt_d, bp_d,
          lns_d, lnb_d, gm_d, out_d):
    nc = tc.nc
    MULT = mybir.AluOpType.mult
    ADD = mybir.AluOpType.add

    wpool = ctx.enter_context(tc.tile_pool(name="weights", bufs=1))
    sm = ctx.enter_context(tc.tile_pool(name="small", bufs=1))
    trp = ctx.enter_context(tc.tile_pool(name="trps", bufs=2, space="PSUM"))
    acc = ctx.enter_context(tc.tile_pool(name="acc", bufs=1, space="PSUM"))

    vt_sb = wpool.tile([128, KT * D], F32, tag="vt")
    ut_sb = wpool.tile([128, KT * D], F32, tag="ut")
    wt_sb = wpool.tile([128, KT * D], F32, tag="wt")

    ha_sb = sm.tile([B_CORE, D], F32, tag="ha")
    al_sb = sm.tile([B_CORE, N_EXP], F32, tag="al")
    bp_sb = sm.tile([N_EXP + 1, D], F32, tag="bp")
    ident = sm.tile([128, 128], F32, tag="ident")
    x_sb = sm.tile([128, KT * B_CORE], F32, tag="x")      # h_A^T tiles
    alt_sb = sm.tile([N_EXP + 1, B_CORE], F32, tag="alt")  # [alpha^T; 1]
    s_sb = sm.tile([B_CORE, D], F32, tag="s")
    st_sb = sm.tile([128, KT * B_CORE], F32, tag="st")    # s^T tiles
    hpre_sb = sm.tile([B_CORE, D], F32, tag="hpre")
    cent_sb = sm.tile([B_CORE, D], F32, tag="cent")
    sq_sb = sm.tile([B_CORE, D], F32, tag="sq")
    y_sb = sm.tile([B_CORE, D], F32, tag="y")
    out_sb = sm.tile([B_CORE, D], F32, tag="out")
    lnsr_sb = sm.tile([B_CORE, D], F32, tag="lnsr")
    lnbr_sb = sm.tile([B_CORE, D], F32, tag="lnbr")
    gmc_sb = sm.tile([B_CORE, 1], F32, tag="gmc")
    psum_acc = [sm.tile([B_CORE, 1], F32, tag=f"pacc{h}", name=f"pacc{h}")
                for h in range(NH)]
    sum_c = sm.tile([B_CORE, 1], F32, tag="sumc")
    negm_c = sm.tile([B_CORE, 1], F32, tag="negmc")
    var_c = sm.tile([B_CORE, 1], F32, tag="varc")
    std_c = sm.tile([B_CORE, 1], F32, tag="stdc")
    istd_c = sm.tile([B_CORE, 1], F32, tag="istdc")
    eps_c = sm.tile([B_CORE, 1], F32, tag="epsc")

    # ---- loads ----
    for w_sb, w_d in ((vt_sb, vt_d), (ut_sb, ut_d), (wt_sb, wt_d)):
        nc.sync.dma_start(
            out=w_sb[:].rearrange("p (i c) -> p i c", i=KT),
            in_=w_d.ap().rearrange("(i p) c -> p i c", p=128),
        )
    nc.sync.dma_start(out=ha_sb[:], in_=ha_d.ap())
    nc.sync.dma_start(out=al_sb[:], in_=al_d.ap())
    nc.sync.dma_start(out=bp_sb[:], in_=bp_d.ap())
    # broadcast loads (replicate row 0 across partitions via step-0 AP)
    nc.gpsimd.dma_start(out=lnsr_sb[:], in_=lns_d.ap().broadcast_to([B_CORE, D]))
    nc.gpsimd.dma_start(out=lnbr_sb[:], in_=lnb_d.ap().broadcast_to([B_CORE, D]))
    nc.gpsimd.dma_start(out=gmc_sb[:], in_=gm_d.ap().broadcast_to([B_CORE, 1]))

    masks.make_identity(nc, ident[:])

    if stage == "loads":
        nc.vector.tensor_copy(out_sb[:], ha_sb[:])
        nc.sync.dma_start(out=out_d.ap(), in_=out_sb[:])
        return

    # ---- transposes: X = h_A^T (per 128-wide a-tile), [alpha^T; 1] ----
    for i in range(KT):
        tp = trp.tile([128, B_CORE], F32, tag="tr", name=f"trx{i}")
        nc.tensor.transpose(tp[:], ha_sb[:, 128 * i:128 * (i + 1)],
                            ident[:B_CORE, :B_CORE])
        nc.vector.tensor_copy(x_sb[:, B_CORE * i:B_CORE * (i + 1)], tp[:])
    tp = trp.tile([128, B_CORE], F32, tag="tr", name="tral")
    nc.tensor.transpose(tp[:N_EXP], al_sb[:], ident[:B_CORE, :B_CORE])
    nc.vector.tensor_copy(alt_sb[:N_EXP], tp[:N_EXP])
    nc.vector.memset(alt_sb[N_EXP:N_EXP + 1, :], 1.0)

    # ---- t = h_A @ V^T ; s = t * repeat(alpha, R) ----
    t_ps = [acc.tile([B_CORE, 512], F32, tag=f"t{h}", name=f"t_ps{h}")
            for h in range(NH)]
    for h in range(NH):
        for i in range(KT):
            nc.tensor.matmul(
                t_ps[h][:],
                _mm_ap(x_sb[:, B_CORE * i:B_CORE * (i + 1)], mode),
                _mm_ap(vt_sb[:, D * i + 512 * h:D * i + 512 * (h + 1)], mode),
                start=(i == 0), stop=(i == KT - 1),
            )
        o3 = s_sb[:, 512 * h:512 * (h + 1)].rearrange(
            "p (n r) -> p n r", r=R_RANK)
        i3 = t_ps[h][:].rearrange("p (n r) -> p n r", r=R_RANK)
        a3 = al_sb[:, 32 * h:32 * (h + 1)].unsqueeze(-1).broadcast_to(
            [B_CORE, 32, R_RANK])
        nc.vector.tensor_mul(o3, i3, a3)

    if stage == "t":
        nc.sync.dma_start(out=out_d.ap(), in_=s_sb[:])
        return

    # ---- s^T tiles ----
    for j in range(KT):
        tp = trp.tile([128, B_CORE], F32, tag="tr", name=f"trs{j}")
        nc.tensor.transpose(tp[:], s_sb[:, 128 * j:128 * (j + 1)],
                            ident[:B_CORE, :B_CORE])
        nc.vector.tensor_copy(st_sb[:, B_CORE * j:B_CORE * (j + 1)], tp[:])

    # ---- h_T = s @ U^T + h_A @ W^T + [alpha,1] @ [bias_pool; b_base] ----
    h_ps = [acc.tile([B_CORE, 512], F32, tag=f"h{h}", name=f"h_ps{h}")
            for h in range(NH)]
    for h in range(NH):
        sl = slice(512 * h, 512 * (h + 1))
        for j in range(KT):
            nc.tensor.matmul(
                h_ps[h][:],
                _mm_ap(st_sb[:, B_CORE * j:B_CORE * (j + 1)], mode),
                _mm_ap(ut_sb[:, D * j + 512 * h:D * j + 512 * (h + 1)], mode),
                start=(j == 0), stop=False,
            )
        for i in range(KT):
            nc.tensor.matmul(
                h_ps[h][:],
                _mm_ap(x_sb[:, B_CORE * i:B_CORE * (i + 1)], mode),
                _mm_ap(wt_sb[:, D * i + 512 * h:D * i + 512 * (h + 1)], mode),
                start=False, stop=False,
            )
        nc.tensor.matmul(h_ps[h][:], _mm_ap(alt_sb[:], mode),
                         _mm_ap(bp_sb[:, sl], mode),
                         start=False, stop=True)
        # h_pre = gamma * h_T + h_A (and per-half row-sums for the mean)
        nc.vector.scalar_tensor_tensor(
            out=hpre_sb[:, sl], in0=h_ps[h][:], scalar=gmc_sb[:],
            in1=ha_sb[:, sl], op0=MULT, op1=ADD,
            accum_out=psum_acc[h][:])

    if stage == "h":
        nc.sync.dma_start(out=out_d.ap(), in_=hpre_sb[:])
        return

    # ---- LayerNorm over the free dim ----
    nc.vector.tensor_add(sum_c[:], psum_acc[0][:], psum_acc[1][:])
    nc.scalar.mul(negm_c[:], sum_c[:], -1.0 / D)
    nc.vector.tensor_scalar_add(cent_sb[:], hpre_sb[:], negm_c[:])
    nc.scalar.activation(sq_sb[:], cent_sb[:],
                         mybir.ActivationFunctionType.Square,
                         accum_out=var_c[:])
    nc.vector.memset(eps_c[:], 1e-5)
    nc.scalar.activation(std_c[:], var_c[:],
                         mybir.ActivationFunctionType.Sqrt,
                         bias=eps_c[:], scale=1.0 / D)
    nc.vector.reciprocal(istd_c[:], std_c[:])
    nc.vector.scalar_tensor_tensor(
        out=y_sb[:], in0=cent_sb[:], scalar=istd_c[:], in1=lnsr_sb[:],
        op0=MULT, op1=MULT)
    nc.vector.tensor_add(out_sb[:], y_sb[:], lnbr_sb[:])

    nc.sync.dma_start(out=out_d.ap(), in_=out_sb[:])


def _prep_in_maps(inputs):
    def f32c(x):
        return np.ascontiguousarray(np.asarray(x, dtype=np.float32))

    h_a = f32c(inputs["h_A"])
    alpha = f32c(inputs["alpha"])
    pool = np.asarray(inputs["pool_vectors"], dtype=np.float32)
    w_base = np.asarray(inputs["W_base"], dtype=np.float32)

    # pool_vectors rows: [U_n (D*R) | V_n (R*D) | bias_n (D)]
    u = pool[:, :D * R_RANK].reshape(N_EXP, D, R_RANK)
    v = pool[:, D * R_RANK:2 * D * R_RANK].reshape(N_EXP, R_RANK, D)
    bias_pool = pool[:, 2 * D * R_RANK:]                    # [64, D]
    bb = np.asarray(inputs["b_base"], dtype=np.float32).reshape(1, D)
    bp = f32c(np.concatenate([bias_pool, bb], axis=0))      # [65, D]
    ut = f32c(u.transpose(0, 2, 1).reshape(N_EXP * R_RANK, D))  # [(n,r), c]
    vt = f32c(v.reshape(N_EXP * R_RANK, D).T)                   # [a, (n,r)]
    wt = f32c(w_base.T)                                          # [a, c]
    lns = f32c(inputs["ln_scale"]).reshape(1, D)
    lnb = f32c(inputs["ln_bias"]).reshape(1, D)
    gm = f32c(inputs["gamma"]).reshape(1, 1)

    in_maps = []
    for k in range(N_CORES):
        rows = slice(B_CORE * k, B_CORE * (k + 1))
        in_maps.append({
            "ha": f32c(h_a[rows]), "al": f32c(alpha[rows]),
            "vt": vt, "ut": ut, "wt": wt, "bp": bp,
            "lns": lns, "lnb": lnb, "gm": gm,
        })
    return in_maps


def get_compiled(mode=None, stage=None):
    key = (mode or MATMUL_MODE, stage or STAGE)
    if key not in _COMPILED:
        _COMPILED[key] = _build(*key)
    return _COMPILED[key]


def kernel(**inputs):
    nc = get_compiled()
    in_maps = _prep_in_maps(inputs)
    res = bass_utils.run_bass_kernel_spmd(
        nc, in_maps, core_ids=list(range(N_CORES)))
    return np.concatenate([r["out"] for r in res.results], axis=0)


# revision 13
# speedup vs baseline: 1.3647x; 1.3647x over previous
"""Trainium2 Bass kernel for the DWA middle layer (moe_routing).

Math (factored form of the reference):
    t     = h_A @ V_flat^T                      # [B, N*R]
    s     = t * repeat(alpha, R, axis=1)        # [B, N*R]
    h_T   = s @ U_flat^T + h_A @ W_base^T + [alpha, 1] @ [bias_pool; b_base]
    out   = LayerNorm(h_A + gamma * h_T) * ln_scale + ln_bias

Sharding: data-parallel over the batch dim (32 rows per core, 8 cores).
Weight matrices are replicated; on the host we only re-lay them out
(transpose/reshape/concat) so the contraction dim lands on SBUF
partitions — all arithmetic runs on device.

All PE matmuls keep the (small) activations stationary and stream the
weight matrices as the moving operand at N=512.
"""

import os
from contextlib import ExitStack

import numpy as np

import concourse.bacc as bacc
import concourse.mybir as mybir
import concourse.tile as tile
from concourse import bass_utils, masks

F32 = mybir.dt.float32
F32R = mybir.dt.float32r

D = 1024          # d_A == d_B
B_CORE = 32       # batch rows per core
N_EXP = 64        # experts
R_RANK = 16       # rank per expert
N_CORES = 8
KT = D // 128     # 8 contraction tiles of 128
NH = D // 512     # 2 moving halves of 512

# "f32r" = raw-fp32 single-pass PE mode (4x faster, slightly relaxed
# multiply precision); "f32" = full two-pass fp32.
MATMUL_MODE = os.environ.get("DWA_MATMUL_MODE", "f32r")
# Debug stage limiter: "loads" | "t" | "h" | "full"
STAGE = os.environ.get("DWA_STAGE", "full")

_COMPILED = {}


def _build(mode, stage="full"):
    nc = bacc.Bacc("TRN2", debug=False, num_devices=N_CORES)
    WDT = F32R if mode == "f32r" else F32

    ha_d = nc.dram_tensor("ha", [B_CORE, D], F32, kind="ExternalInput")
    al_d = nc.dram_tensor("al", [B_CORE, N_EXP], F32, kind="ExternalInput")
    vt_d = nc.dram_tensor("vt", [D, D], WDT, kind="ExternalInput")    # [a, nr]
    ut_d = nc.dram_tensor("ut", [D, D], WDT, kind="ExternalInput")    # [nr, c]
    wt_d = nc.dram_tensor("wt", [D, D], WDT, kind="ExternalInput")    # [a, c]
    bp_d = nc.dram_tensor("bp", [N_EXP + 1, D], WDT, kind="ExternalInput")
    lns_d = nc.dram_tensor("lns", [1, D], F32, kind="ExternalInput")
    lnb_d = nc.dram_tensor("lnb", [1, D], F32, kind="ExternalInput")
    gm_d = nc.dram_tensor("gm", [1, 1], F32, kind="ExternalInput")
    out_d = nc.dram_tensor("out", [B_CORE, D], F32, kind="ExternalOutput")

    with ExitStack() as ctx:
        tc = ctx.enter_context(tile.TileContext(nc))
        _emit(ctx, tc, WDT, stage, ha_d, al_d, vt_d, ut_d, wt_d, bp_d,
              lns_d, lnb_d, gm_d, out_d)

    nc.compile()
    return nc


def _emit(ctx, tc, WDT, stage, ha_d, al_d, vt_d, ut_d, wt_d, bp_d,
          lns_d, lnb_d, gm_d, out_d):
    nc = tc.nc
    MULT = mybir.AluOpType.mult
    ADD = mybir.AluOpType.add

    wpool = ctx.enter_context(tc.tile_pool(name="weights", bufs=1))
    sm = ctx.enter_context(tc.tile_pool(name="small", bufs=1))
    trp = ctx.enter_context(tc.tile_pool(name="trps", bufs=2, space="PSUM"))
    acc = ctx.enter_context(tc.tile_pool(name="acc", bufs=1, space="PSUM"))

    vt_sb = wpool.tile([128, KT * D], WDT, tag="vt")
    ut_sb = wpool.tile([128, KT * D], WDT, tag="ut")
    wt_sb = wpool.tile([128, KT * D], WDT, tag="wt")

    ha_sb = sm.tile([B_CORE, D], F32, tag="ha")
    al_sb = sm.tile([B_CORE, N_EXP + 1], F32, tag="al")  # [alpha | 1]
    bp_sb = sm.tile([N_EXP + 1, D], WDT, tag="bp")
    ident = sm.tile([128, 128], F32, tag="ident")
    x_sb = sm.tile([128, KT * B_CORE], WDT, tag="x")      # h_A^T tiles
    alt_sb = sm.tile([N_EXP + 1, B_CORE], WDT, tag="alt")  # [alpha^T; 1]
    s_sb = sm.tile([B_CORE, D], F32, tag="s")
    st_sb = sm.tile([128, KT * B_CORE], WDT, tag="st")    # s^T tiles
    hpre_sb = sm.tile([B_CORE, D], F32, tag="hpre")
    cent_sb = sm.tile([B_CORE, D], F32, tag="cent")
    sq_sb = sm.tile([B_CORE, D], F32, tag="sq")
    y_sb = sm.tile([B_CORE, D], F32, tag="y")
    out_sb = sm.tile([B_CORE, D], F32, tag="out")
    lnsr_sb = sm.tile([B_CORE, D], F32, tag="lnsr")
    lnbr_sb = sm.tile([B_CORE, D], F32, tag="lnbr")
    gmc_sb = sm.tile([B_CORE, 1], F32, tag="gmc")
    psum_acc = [sm.tile([B_CORE, 1], F32, tag=f"pacc{h}", name=f"pacc{h}")
                for h in range(NH)]
    sum_c = sm.tile([B_CORE, 1], F32, tag="sumc")
    negm_c = sm.tile([B_CORE, 1], F32, tag="negmc")
    var_c = sm.tile([B_CORE, 1], F32, tag="varc")
    std_c = sm.tile([B_CORE, 1], F32, tag="stdc")
    istd_c = sm.tile([B_CORE, 1], F32, tag="istdc")
    eps_c = sm.tile([B_CORE, 1], F32, tag="epsc")

    # ---- loads ----
    for w_sb, w_d in ((vt_sb, vt_d), (ut_sb, ut_d), (wt_sb, wt_d)):
        nc.sync.dma_start(
            out=w_sb[:].rearrange("p (i c) -> p i c", i=KT),
            in_=w_d.ap().rearrange("(i p) c -> p i c", p=128),
        )
    nc.sync.dma_start(out=ha_sb[:], in_=ha_d.ap())
    nc.sync.dma_start(out=al_sb[:, :N_EXP], in_=al_d.ap())
    nc.vector.memset(al_sb[:, N_EXP:N_EXP + 1], 1.0)
    nc.sync.dma_start(out=bp_sb[:], in_=bp_d.ap())
    # broadcast loads (replicate row 0 across partitions via step-0 AP)
    nc.gpsimd.dma_start(out=lnsr_sb[:], in_=lns_d.ap().broadcast_to([B_CORE, D]))
    nc.gpsimd.dma_start(out=lnbr_sb[:], in_=lnb_d.ap().broadcast_to([B_CORE, D]))
    nc.gpsimd.dma_start(out=gmc_sb[:], in_=gm_d.ap().broadcast_to([B_CORE, 1]))

    masks.make_identity(nc, ident[:])

    if stage == "loads":
        nc.vector.tensor_copy(out_sb[:], ha_sb[:])
        nc.sync.dma_start(out=out_d.ap(), in_=out_sb[:])
        return

    # ---- transposes: X = h_A^T (per 128-wide a-tile), [alpha^T; 1] ----
    for i in range(KT):
        tp = trp.tile([128, B_CORE], F32, tag="tr", name=f"trx{i}")
        nc.tensor.transpose(tp[:], ha_sb[:, 128 * i:128 * (i + 1)],
                            ident[:B_CORE, :B_CORE])
        nc.vector.tensor_copy(x_sb[:, B_CORE * i:B_CORE * (i + 1)], tp[:])
    tp = trp.tile([128, B_CORE], F32, tag="tr", name="tral")
    nc.tensor.transpose(tp[:N_EXP + 1], al_sb[:], ident[:B_CORE, :B_CORE])
    nc.vector.tensor_copy(alt_sb[:], tp[:N_EXP + 1])

    # ---- t = h_A @ V^T ; s = t * repeat(alpha, R) ----
    t_ps = [acc.tile([B_CORE, 512], F32, tag=f"t{h}", name=f"t_ps{h}")
            for h in range(NH)]
    for h in range(NH):
        for i in range(KT):
            nc.tensor.matmul(
                t_ps[h][:],
                x_sb[:, B_CORE * i:B_CORE * (i + 1)],
                vt_sb[:, D * i + 512 * h:D * i + 512 * (h + 1)],
                start=(i == 0), stop=(i == KT - 1),
            )
        o3 = s_sb[:, 512 * h:512 * (h + 1)].rearrange(
            "p (n r) -> p n r", r=R_RANK)
        i3 = t_ps[h][:].rearrange("p (n r) -> p n r", r=R_RANK)
        a3 = al_sb[:, 32 * h:32 * (h + 1)].unsqueeze(-1).broadcast_to(
            [B_CORE, 32, R_RANK])
        nc.vector.tensor_mul(o3, i3, a3)

    if stage == "t":
        nc.sync.dma_start(out=out_d.ap(), in_=s_sb[:])
        return

    # ---- s^T tiles ----
    for j in range(KT):
        tp = trp.tile([128, B_CORE], F32, tag="tr", name=f"trs{j}")
        nc.tensor.transpose(tp[:], s_sb[:, 128 * j:128 * (j + 1)],
                            ident[:B_CORE, :B_CORE])
        nc.vector.tensor_copy(st_sb[:, B_CORE * j:B_CORE * (j + 1)], tp[:])

    # ---- h_T = s @ U^T + h_A @ W^T + [alpha,1] @ [bias_pool; b_base] ----
    h_ps = [acc.tile([B_CORE, 512], F32, tag=f"h{h}", name=f"h_ps{h}")
            for h in range(NH)]
    for h in range(NH):
        sl = slice(512 * h, 512 * (h + 1))
        for j in range(KT):
            nc.tensor.matmul(
                h_ps[h][:],
                st_sb[:, B_CORE * j:B_CORE * (j + 1)],
                ut_sb[:, D * j + 512 * h:D * j + 512 * (h + 1)],
                start=(j == 0), stop=False,
            )
        for i in range(KT):
            nc.tensor.matmul(
                h_ps[h][:],
                x_sb[:, B_CORE * i:B_CORE * (i + 1)],
                wt_sb[:, D * i + 512 * h:D * i + 512 * (h + 1)],
                start=False, stop=False,
            )
        nc.tensor.matmul(h_ps[h][:], alt_sb[:], bp_sb[:, sl],
                         start=False, stop=True)
        # h_pre = gamma * h_T + h_A (and per-half row-sums for the mean)
        nc.vector.scalar_tensor_tensor(
            out=hpre_sb[:, sl], in0=h_ps[h][:], scalar=gmc_sb[:],
            in1=ha_sb[:, sl], op0=MULT, op1=ADD,
            accum_out=psum_acc[h][:])

    if stage == "h":
        nc.sync.dma_start(out=out_d.ap(), in_=hpre_sb[:])
        return

    # ---- LayerNorm over the free dim ----
    nc.vector.tensor_add(sum_c[:], psum_acc[0][:], psum_acc[1][:])
    nc.scalar.mul(negm_c[:], sum_c[:], -1.0 / D)
    nc.vector.tensor_scalar_add(cent_sb[:], hpre_sb[:], negm_c[:])
    nc.scalar.activation(sq_sb[:], cent_sb[:],
                         mybir.ActivationFunctionType.Square,
                         accum_out=var_c[:])
    nc.vector.memset(eps_c[:], 1e-5)
    nc.scalar.activation(std_c[:], var_c[:],
                         mybir.ActivationFunctionType.Sqrt,
                         bias=eps_c[:], scale=1.0 / D)
    nc.vector.reciprocal(istd_c[:], std_c[:])
    nc.vector.scalar_tensor_tensor(
        out=y_sb[:], in0=cent_sb[:], scalar=istd_c[:], in1=lnsr_sb[:],
        op0=MULT, op1=MULT)
    nc.vector.tensor_add(out_sb[:], y_sb[:], lnbr_sb[:])

    nc.sync.dma_start(out=out_d.ap(), in_=out_sb[:])


def _prep_in_maps(inputs):
    def f32c(x):
        return np.ascontiguousarray(np.asarray(x, dtype=np.float32))

    h_a = f32c(inputs["h_A"])
    alpha = f32c(inputs["alpha"])
    pool = np.asarray(inputs["pool_vectors"], dtype=np.float32)
    w_base = np.asarray(inputs["W_base"], dtype=np.float32)

    # pool_vectors rows: [U_n (D*R) | V_n (R*D) | bias_n (D)]
    u = pool[:, :D * R_RANK].reshape(N_EXP, D, R_RANK)
    v = pool[:, D * R_RANK:2 * D * R_RANK].reshape(N_EXP, R_RANK, D)
    bias_pool = pool[:, 2 * D * R_RANK:]                    # [64, D]
    bb = np.asarray(inputs["b_base"], dtype=np.float32).reshape(1, D)
    bp = f32c(np.concatenate([bias_pool, bb], axis=0))      # [65, D]
    ut = f32c(u.transpose(0, 2, 1).reshape(N_EXP * R_RANK, D))  # [(n,r), c]
    vt = f32c(v.reshape(N_EXP * R_RANK, D).T)                   # [a, (n,r)]
    wt = f32c(w_base.T)                                          # [a, c]
    lns = f32c(inputs["ln_scale"]).reshape(1, D)
    lnb = f32c(inputs["ln_bias"]).reshape(1, D)
    gm = f32c(inputs["gamma"]).reshape(1, 1)

    in_maps = []
    for k in range(N_CORES):
        rows = slice(B_CORE * k, B_CORE * (k + 1))
        in_maps.append({
            "ha": f32c(h_a[rows]), "al": f32c(alpha[rows]),
            "vt": vt, "ut": ut, "wt": wt, "bp": bp,
            "lns": lns, "lnb": lnb, "gm": gm,
        })
    return in_maps


def get_compiled(mode=None, stage=None):
    key = (mode or MATMUL_MODE, stage or STAGE)
    if key not in _COMPILED:
        _COMPILED[key] = _build(*key)
    return _COMPILED[key]


def kernel(**inputs):
    nc = get_compiled()
    in_maps = _prep_in_maps(inputs)
    res = bass_utils.run_bass_kernel_spmd(
        nc, in_maps, core_ids=list(range(N_CORES)))
    return np.concatenate([r["out"] for r in res.results], axis=0)


# revision 15
# speedup vs baseline: 1.6730x; 1.2259x over previous
"""Trainium2 Bass kernel for the DWA middle layer (moe_routing).

Math (factored form of the reference):
    t     = h_A @ V_flat^T                      # [B, N*R]
    s     = t * repeat(alpha, R, axis=1)        # [B, N*R]
    h_T   = s @ U_flat^T + h_A @ W_base^T + [alpha, 1] @ [bias_pool; b_base]
    out   = LayerNorm(h_A + gamma * h_T) * ln_scale + ln_bias

Sharding: data-parallel over the batch dim (32 rows per core, 8 cores).
Weight matrices are replicated; on the host we only re-lay them out
(transpose/reshape/concat into the SBUF-native partition-major layout)
so the contraction dim lands on SBUF partitions — all arithmetic runs
on device.

All PE matmuls keep the (small) activations stationary and stream the
weight matrices as the moving operand at N=512.  Weight DMAs are issued
in 512KB k-tile chunks interleaved with the matmuls that consume them,
so the PE pipeline runs under the (HBM-bound) weight stream.
"""

import os
from contextlib import ExitStack

import numpy as np

import concourse.bacc as bacc
import concourse.mybir as mybir
import concourse.tile as tile
from concourse import bass_utils, masks

F32 = mybir.dt.float32
F32R = mybir.dt.float32r

D = 1024          # d_A == d_B
B_CORE = 32       # batch rows per core
N_EXP = 64        # experts
R_RANK = 16       # rank per expert
N_CORES = 8
KT = D // 128     # 8 contraction tiles of 128
NH = D // 512     # 2 moving halves of 512

# "f32r" = raw-fp32 single-pass PE mode (faster, slightly relaxed
# multiply precision); "f32" = full two-pass fp32.
MATMUL_MODE = os.environ.get("DWA_MATMUL_MODE", "f32r")
STAGE = os.environ.get("DWA_STAGE", "full")

_COMPILED = {}


def _build(mode, stage="full"):
    nc = bacc.Bacc("TRN2", debug=False, num_devices=N_CORES)
    WDT = F32R if mode == "f32r" else F32

    ha_d = nc.dram_tensor("ha", [B_CORE, D], F32, kind="ExternalInput")
    al_d = nc.dram_tensor("al", [B_CORE, N_EXP], F32, kind="ExternalInput")
    # weights in SBUF-native partition-major layout [128, KT*1024]
    vt_d = nc.dram_tensor("vt", [128, KT * D], WDT, kind="ExternalInput")
    ut_d = nc.dram_tensor("ut", [128, KT * D], WDT, kind="ExternalInput")
    wt_d = nc.dram_tensor("wt", [128, KT * D], WDT, kind="ExternalInput")
    bp_d = nc.dram_tensor("bp", [N_EXP + 1, D], WDT, kind="ExternalInput")
    lns_d = nc.dram_tensor("lns", [1, D], F32, kind="ExternalInput")
    lnb_d = nc.dram_tensor("lnb", [1, D], F32, kind="ExternalInput")
    gm_d = nc.dram_tensor("gm", [1, 1], F32, kind="ExternalInput")
    out_d = nc.dram_tensor("out", [B_CORE, D], F32, kind="ExternalOutput")

    with ExitStack() as ctx:
        tc = ctx.enter_context(tile.TileContext(nc))
        _emit(ctx, tc, WDT, stage, ha_d, al_d, vt_d, ut_d, wt_d, bp_d,
              lns_d, lnb_d, gm_d, out_d)

    nc.compile()
    return nc


def _emit(ctx, tc, WDT, stage, ha_d, al_d, vt_d, ut_d, wt_d, bp_d,
          lns_d, lnb_d, gm_d, out_d):
    nc = tc.nc
    MULT = mybir.AluOpType.mult
    ADD = mybir.AluOpType.add
    SQRT = mybir.ActivationFunctionType.Sqrt

    wpool = ctx.enter_context(tc.tile_pool(name="weights", bufs=1))
    sm = ctx.enter_context(tc.tile_pool(name="small", bufs=1))
    trp = ctx.enter_context(tc.tile_pool(name="trps", bufs=2, space="PSUM"))
    acc = ctx.enter_context(tc.tile_pool(name="acc", bufs=1, space="PSUM"))

    vt_sb = wpool.tile([128, KT * D], WDT, tag="vt")
    ut_sb = wpool.tile([128, KT * D], WDT, tag="ut")
    wt_sb = wpool.tile([128, KT * D], WDT, tag="wt")

    ha_sb = sm.tile([B_CORE, D], F32, tag="ha")
    al_sb = sm.tile([B_CORE, N_EXP + 1], F32, tag="al")  # [alpha | 1]
    bp_sb = sm.tile([N_EXP + 1, D], WDT, tag="bp")
    ident = sm.tile([128, 128], F32, tag="ident")
    x_sb = sm.tile([128, KT * B_CORE], WDT, tag="x")      # h_A^T tiles
    alt_sb = sm.tile([N_EXP + 1, B_CORE], WDT, tag="alt")  # [alpha^T; 1]
    s_sb = sm.tile([B_CORE, D], F32, tag="s")
    st_sb = sm.tile([128, KT * B_CORE], WDT, tag="st")    # s^T tiles
    hpre_sb = sm.tile([B_CORE, D], F32, tag="hpre")
    sq_sb = sm.tile([B_CORE, D], F32, tag="sq")
    y_sb = sm.tile([B_CORE, D], F32, tag="y")
    t2_sb = sm.tile([B_CORE, D], F32, tag="t2")
    out_sb = sm.tile([B_CORE, D], F32, tag="out")
    lnsr_sb = sm.tile([B_CORE, D], F32, tag="lnsr")
    lnbr_sb = sm.tile([B_CORE, D], F32, tag="lnbr")
    gmc_sb = sm.tile([B_CORE, 1], F32, tag="gmc")
    sum_h = [sm.tile([B_CORE, 1], F32, tag=f"sumh{h}", name=f"sumh{h}")
             for h in range(NH)]
    ssq_h = [sm.tile([B_CORE, 1], F32, tag=f"ssqh{h}", name=f"ssqh{h}")
             for h in range(NH)]
    sum_c = sm.tile([B_CORE, 1], F32, tag="sumc")
    m_c = sm.tile([B_CORE, 1], F32, tag="mc")
    msq_c = sm.tile([B_CORE, 1], F32, tag="msqc")
    ssq_c = sm.tile([B_CORE, 1], F32, tag="ssqc")
    var_c = sm.tile([B_CORE, 1], F32, tag="varc")
    std_c = sm.tile([B_CORE, 1], F32, tag="stdc")
    istd_c = sm.tile([B_CORE, 1], F32, tag="istdc")
    nmi_c = sm.tile([B_CORE, 1], F32, tag="nmic")
    eps_c = sm.tile([B_CORE, 1], F32, tag="epsc")
    warm_c = sm.tile([B_CORE, 1], F32, tag="warmc")

    # ---- small loads first ----
    nc.sync.dma_start(out=ha_sb[:], in_=ha_d.ap())
    nc.sync.dma_start(out=al_sb[:, :N_EXP], in_=al_d.ap())
    nc.sync.dma_start(out=bp_sb[:], in_=bp_d.ap())
    nc.vector.memset(al_sb[:, N_EXP:N_EXP + 1], 1.0)
    nc.vector.memset(eps_c[:], 1e-5)
    # broadcast loads (replicate row 0 across partitions via step-0 AP)
    nc.gpsimd.dma_start(out=gmc_sb[:], in_=gm_d.ap().broadcast_to([B_CORE, 1]))
    nc.gpsimd.dma_start(out=lnsr_sb[:], in_=lns_d.ap().broadcast_to([B_CORE, D]))
    nc.gpsimd.dma_start(out=lnbr_sb[:], in_=lnb_d.ap().broadcast_to([B_CORE, D]))

    masks.make_identity(nc, ident[:])
    # preload the Sqrt ACT table off the critical path
    nc.scalar.activation(warm_c[:], eps_c[:], SQRT, bias=eps_c[:], scale=1.0)

    if stage == "loads":
        nc.vector.tensor_copy(out_sb[:], ha_sb[:])
        nc.sync.dma_start(out=out_d.ap(), in_=out_sb[:])
        return

    # ---- transposes: X = h_A^T (per 128-wide a-tile), [alpha^T; 1] ----
    for i in range(KT):
        tp = trp.tile([128, B_CORE], F32, tag="tr", name=f"trx{i}")
        nc.tensor.transpose(tp[:], ha_sb[:, 128 * i:128 * (i + 1)],
                            ident[:B_CORE, :B_CORE])
        nc.vector.tensor_copy(x_sb[:, B_CORE * i:B_CORE * (i + 1)], tp[:])
    tp = trp.tile([128, B_CORE], F32, tag="tr", name="tral")
    nc.tensor.transpose(tp[:N_EXP + 1], al_sb[:], ident[:B_CORE, :B_CORE])
    nc.vector.tensor_copy(alt_sb[:], tp[:N_EXP + 1])

    # ---- t = h_A @ V^T ; s = t * repeat(alpha, R) ----
    t_ps = [acc.tile([B_CORE, 512], F32, tag=f"t{h}", name=f"t_ps{h}")
            for h in range(NH)]
    for i in range(KT):
        csl = slice(D * i, D * (i + 1))
        nc.sync.dma_start(out=vt_sb[:, csl], in_=vt_d.ap()[:, csl])
        for h in range(NH):
            nc.tensor.matmul(
                t_ps[h][:],
                x_sb[:, B_CORE * i:B_CORE * (i + 1)],
                vt_sb[:, D * i + 512 * h:D * i + 512 * (h + 1)],
                start=(i == 0), stop=(i == KT - 1),
            )
    for h in range(NH):
        o3 = s_sb[:, 512 * h:512 * (h + 1)].rearrange(
            "p (n r) -> p n r", r=R_RANK)
        i3 = t_ps[h][:].rearrange("p (n r) -> p n r", r=R_RANK)
        a3 = al_sb[:, 32 * h:32 * (h + 1)].unsqueeze(-1).broadcast_to(
            [B_CORE, 32, R_RANK])
        nc.vector.tensor_mul(o3, i3, a3)

    if stage == "t":
        nc.sync.dma_start(out=out_d.ap(), in_=s_sb[:])
        return

    # ---- s^T tiles ----
    for j in range(KT):
        tp = trp.tile([128, B_CORE], F32, tag="tr", name=f"trs{j}")
        nc.tensor.transpose(tp[:], s_sb[:, 128 * j:128 * (j + 1)],
                            ident[:B_CORE, :B_CORE])
        nc.vector.tensor_copy(st_sb[:, B_CORE * j:B_CORE * (j + 1)], tp[:])

    # ---- h_T = s @ U^T + h_A @ W^T + [alpha,1] @ [bias_pool; b_base] ----
    h_ps = [acc.tile([B_CORE, 512], F32, tag=f"h{h}", name=f"h_ps{h}")
            for h in range(NH)]
    for j in range(KT):
        csl = slice(D * j, D * (j + 1))
        nc.sync.dma_start(out=ut_sb[:, csl], in_=ut_d.ap()[:, csl])
        for h in range(NH):
            nc.tensor.matmul(
                h_ps[h][:],
                st_sb[:, B_CORE * j:B_CORE * (j + 1)],
                ut_sb[:, D * j + 512 * h:D * j + 512 * (h + 1)],
                start=(j == 0), stop=False,
            )
    for i in range(KT):
        csl = slice(D * i, D * (i + 1))
        nc.sync.dma_start(out=wt_sb[:, csl], in_=wt_d.ap()[:, csl])
        for h in range(NH):
            nc.tensor.matmul(
                h_ps[h][:],
                x_sb[:, B_CORE * i:B_CORE * (i + 1)],
                wt_sb[:, D * i + 512 * h:D * i + 512 * (h + 1)],
                start=False, stop=False,
            )
    for h in range(NH):
        sl = slice(512 * h, 512 * (h + 1))
        nc.tensor.matmul(h_ps[h][:], alt_sb[:], bp_sb[:, sl],
                         start=False, stop=True)
        # h_pre = gamma * h_T + h_A, with row-sums for the mean
        nc.vector.scalar_tensor_tensor(
            out=hpre_sb[:, sl], in0=h_ps[h][:], scalar=gmc_sb[:],
            in1=ha_sb[:, sl], op0=MULT, op1=ADD,
            accum_out=sum_h[h][:])
        # row-sums of squares for the variance (overlaps the other half)
        nc.vector.scalar_tensor_tensor(
            out=sq_sb[:, sl], in0=hpre_sb[:, sl], scalar=1.0,
            in1=hpre_sb[:, sl], op0=MULT, op1=MULT,
            accum_out=ssq_h[h][:])

    if stage == "h":
        nc.sync.dma_start(out=out_d.ap(), in_=hpre_sb[:])
        return

    # ---- LayerNorm via E[x^2] - E[x]^2 ----
    nc.vector.tensor_add(sum_c[:], sum_h[0][:], sum_h[1][:])
    nc.vector.tensor_add(ssq_c[:], ssq_h[0][:], ssq_h[1][:])
    nc.scalar.mul(m_c[:], sum_c[:], 1.0 / D)
    nc.vector.tensor_mul(msq_c[:], m_c[:], m_c[:])
    nc.vector.scalar_tensor_tensor(
        out=var_c[:], in0=ssq_c[:], scalar=1.0 / D, in1=msq_c[:],
        op0=MULT, op1=mybir.AluOpType.subtract)
    nc.scalar.activation(std_c[:], var_c[:], SQRT, bias=eps_c[:], scale=1.0)
    nc.vector.reciprocal(istd_c[:], std_c[:])
    # out = hpre*istd*lns + (lnb - m*istd*lns)
    nc.vector.tensor_mul(nmi_c[:], m_c[:], istd_c[:])
    nc.scalar.mul(nmi_c[:], nmi_c[:], -1.0)
    nc.vector.scalar_tensor_tensor(
        out=t2_sb[:], in0=lnsr_sb[:], scalar=nmi_c[:], in1=lnbr_sb[:],
        op0=MULT, op1=ADD)
    nc.vector.scalar_tensor_tensor(
        out=y_sb[:], in0=hpre_sb[:], scalar=istd_c[:], in1=lnsr_sb[:],
        op0=MULT, op1=MULT)
    nc.vector.tensor_add(out_sb[:], y_sb[:], t2_sb[:])

    nc.sync.dma_start(out=out_d.ap(), in_=out_sb[:])


def _to_sbuf_layout(a):
    """[KT*128, D] logical -> [128, KT*D] partition-major."""
    return np.ascontiguousarray(
        a.reshape(KT, 128, D).transpose(1, 0, 2).reshape(128, KT * D))


def _prep_in_maps(inputs):
    def f32c(x):
        return np.ascontiguousarray(np.asarray(x, dtype=np.float32))

    h_a = f32c(inputs["h_A"])
    alpha = f32c(inputs["alpha"])
    pool = np.asarray(inputs["pool_vectors"], dtype=np.float32)
    w_base = np.asarray(inputs["W_base"], dtype=np.float32)

    # pool_vectors rows: [U_n (D*R) | V_n (R*D) | bias_n (D)]
    u = pool[:, :D * R_RANK].reshape(N_EXP, D, R_RANK)
    v = pool[:, D * R_RANK:2 * D * R_RANK].reshape(N_EXP, R_RANK, D)
    bias_pool = pool[:, 2 * D * R_RANK:]                    # [64, D]
    bb = np.asarray(inputs["b_base"], dtype=np.float32).reshape(1, D)
    bp = f32c(np.concatenate([bias_pool, bb], axis=0))      # [65, D]
    ut = _to_sbuf_layout(
        f32c(u.transpose(0, 2, 1).reshape(N_EXP * R_RANK, D)))  # [(n,r), c]
    vt = _to_sbuf_layout(f32c(v.reshape(N_EXP * R_RANK, D).T))  # [a, (n,r)]
    wt = _to_sbuf_layout(f32c(w_base.T))                        # [a, c]
    lns = f32c(inputs["ln_scale"]).reshape(1, D)
    lnb = f32c(inputs["ln_bias"]).reshape(1, D)
    gm = f32c(inputs["gamma"]).reshape(1, 1)

    in_maps = []
    for k in range(N_CORES):
        rows = slice(B_CORE * k, B_CORE * (k + 1))
        in_maps.append({
            "ha": f32c(h_a[rows]), "al": f32c(alpha[rows]),
            "vt": vt, "ut": ut, "wt": wt, "bp": bp,
            "lns": lns, "lnb": lnb, "gm": gm,
        })
    return in_maps


def get_compiled(mode=None, stage=None):
    key = (mode or MATMUL_MODE, stage or STAGE)
    if key not in _COMPILED:
        _COMPILED[key] = _build(*key)
    return _COMPILED[key]


def kernel(**inputs):
    nc = get_compiled()
    in_maps = _prep_in_maps(inputs)
    res = bass_utils.run_bass_kernel_spmd(
        nc, in_maps, core_ids=list(range(N_CORES)))
    return np.concatenate([r["out"] for r in res.results], axis=0)


# revision 17
# speedup vs baseline: 1.7650x; 1.0550x over previous
"""Trainium2 Bass kernel for the DWA middle layer (moe_routing).

Math (factored form of the reference):
    t     = h_A @ V_flat^T                      # [B, N*R]
    s     = t * repeat(alpha, R, axis=1)        # [B, N*R]
    h_T   = s @ U_flat^T + h_A @ W_base^T + [alpha, 1] @ [bias_pool; b_base]
    out   = LayerNorm(h_A + gamma * h_T) * ln_scale + ln_bias

Sharding: data-parallel over the batch dim (32 rows per core, 8 cores).
Weight matrices are replicated; on the host we only re-lay them out
(transpose/reshape/concat into the SBUF-native partition-major layout)
so the contraction dim lands on SBUF partitions — all arithmetic runs
on device.

All PE matmuls keep the (small) activations stationary and stream the
weight matrices as the moving operand at N=512.  Weight DMAs are issued
in 512KB k-tile chunks interleaved with the matmuls that consume them,
so the PE pipeline runs under the (HBM-bound) weight stream.
"""

import os
from contextlib import ExitStack

import numpy as np

import concourse.bacc as bacc
import concourse.mybir as mybir
import concourse.tile as tile
from concourse import bass_utils, masks

F32 = mybir.dt.float32
F32R = mybir.dt.float32r

D = 1024          # d_A == d_B
B_CORE = 32       # batch rows per core
N_EXP = 64        # experts
R_RANK = 16       # rank per expert
N_CORES = 8
KT = D // 128     # 8 contraction tiles of 128
NH = D // 512     # 2 moving halves of 512

# "f32r" = raw-fp32 single-pass PE mode (faster, slightly relaxed
# multiply precision); "f32" = full two-pass fp32.
MATMUL_MODE = os.environ.get("DWA_MATMUL_MODE", "f32r")
STAGE = os.environ.get("DWA_STAGE", "full")

_COMPILED = {}


def _build(mode, stage="full"):
    nc = bacc.Bacc("TRN2", debug=False, num_devices=N_CORES)
    WDT = F32R if mode == "f32r" else F32

    ha_d = nc.dram_tensor("ha", [B_CORE, D], F32, kind="ExternalInput")
    al_d = nc.dram_tensor("al", [B_CORE, N_EXP], F32, kind="ExternalInput")
    # weights in SBUF-native partition-major layout [128, KT*1024]
    vt_d = nc.dram_tensor("vt", [128, KT * D], WDT, kind="ExternalInput")
    ut_d = nc.dram_tensor("ut", [128, KT * D], WDT, kind="ExternalInput")
    wt_d = nc.dram_tensor("wt", [128, KT * D], WDT, kind="ExternalInput")
    bp_d = nc.dram_tensor("bp", [N_EXP + 1, D], WDT, kind="ExternalInput")
    lns_d = nc.dram_tensor("lns", [1, D], F32, kind="ExternalInput")
    lnb_d = nc.dram_tensor("lnb", [1, D], F32, kind="ExternalInput")
    gm_d = nc.dram_tensor("gm", [1, 1], F32, kind="ExternalInput")
    out_d = nc.dram_tensor("out", [B_CORE, D], F32, kind="ExternalOutput")

    with ExitStack() as ctx:
        tc = ctx.enter_context(tile.TileContext(nc))
        _emit(ctx, tc, WDT, stage, ha_d, al_d, vt_d, ut_d, wt_d, bp_d,
              lns_d, lnb_d, gm_d, out_d)

    nc.compile()
    return nc


def _emit(ctx, tc, WDT, stage, ha_d, al_d, vt_d, ut_d, wt_d, bp_d,
          lns_d, lnb_d, gm_d, out_d):
    nc = tc.nc
    MULT = mybir.AluOpType.mult
    ADD = mybir.AluOpType.add
    SQRT = mybir.ActivationFunctionType.Sqrt

    wpool = ctx.enter_context(tc.tile_pool(name="weights", bufs=1))
    sm = ctx.enter_context(tc.tile_pool(name="small", bufs=1))
    trp = ctx.enter_context(tc.tile_pool(name="trps", bufs=2, space="PSUM"))
    acc = ctx.enter_context(tc.tile_pool(name="acc", bufs=1, space="PSUM"))

    vt_sb = wpool.tile([128, KT * D], WDT, tag="vt")
    ut_sb = wpool.tile([128, KT * D], WDT, tag="ut")
    wt_sb = wpool.tile([128, KT * D], WDT, tag="wt")

    ha_sb = sm.tile([B_CORE, D], F32, tag="ha")
    al_sb = sm.tile([B_CORE, N_EXP + 1], F32, tag="al")  # [alpha | 1]
    bp_sb = sm.tile([N_EXP + 1, D], WDT, tag="bp")
    ident = sm.tile([128, 128], F32, tag="ident")
    x_sb = sm.tile([128, KT * B_CORE], WDT, tag="x")      # h_A^T tiles
    alt_sb = sm.tile([N_EXP + 1, B_CORE], WDT, tag="alt")  # [alpha^T; 1]
    s_sb = sm.tile([B_CORE, D], F32, tag="s")
    st_sb = sm.tile([128, KT * B_CORE], WDT, tag="st")    # s^T tiles
    hpre_sb = sm.tile([B_CORE, D], F32, tag="hpre")
    sq_sb = sm.tile([B_CORE, D], F32, tag="sq")
    y_sb = sm.tile([B_CORE, D], F32, tag="y")
    t2_sb = sm.tile([B_CORE, D], F32, tag="t2")
    out_sb = sm.tile([B_CORE, D], F32, tag="out")
    lnsr_sb = sm.tile([B_CORE, D], F32, tag="lnsr")
    lnbr_sb = sm.tile([B_CORE, D], F32, tag="lnbr")
    gmc_sb = sm.tile([B_CORE, 1], F32, tag="gmc")
    sum_h = [sm.tile([B_CORE, 1], F32, tag=f"sumh{h}", name=f"sumh{h}")
             for h in range(NH)]
    ssq_h = [sm.tile([B_CORE, 1], F32, tag=f"ssqh{h}", name=f"ssqh{h}")
             for h in range(NH)]
    sum_c = sm.tile([B_CORE, 1], F32, tag="sumc")
    m_c = sm.tile([B_CORE, 1], F32, tag="mc")
    msq_c = sm.tile([B_CORE, 1], F32, tag="msqc")
    ssq_c = sm.tile([B_CORE, 1], F32, tag="ssqc")
    var_c = sm.tile([B_CORE, 1], F32, tag="varc")
    std_c = sm.tile([B_CORE, 1], F32, tag="stdc")
    istd_c = sm.tile([B_CORE, 1], F32, tag="istdc")
    nmi_c = sm.tile([B_CORE, 1], F32, tag="nmic")
    eps_c = sm.tile([B_CORE, 1], F32, tag="epsc")
    warm_c = sm.tile([B_CORE, 1], F32, tag="warmc")

    # ---- small loads first ----
    nc.sync.dma_start(out=ha_sb[:], in_=ha_d.ap())
    nc.sync.dma_start(out=al_sb[:, :N_EXP], in_=al_d.ap())
    nc.sync.dma_start(out=bp_sb[:], in_=bp_d.ap())
    nc.vector.memset(al_sb[:, N_EXP:N_EXP + 1], 1.0)
    nc.vector.memset(eps_c[:], 1e-5)
    # broadcast loads (replicate row 0 across partitions via step-0 AP)
    nc.gpsimd.dma_start(out=gmc_sb[:], in_=gm_d.ap().broadcast_to([B_CORE, 1]))
    nc.gpsimd.dma_start(out=lnsr_sb[:], in_=lns_d.ap().broadcast_to([B_CORE, D]))
    nc.gpsimd.dma_start(out=lnbr_sb[:], in_=lnb_d.ap().broadcast_to([B_CORE, D]))

    masks.make_identity(nc, ident[:])
    # preload the Sqrt ACT table off the critical path
    nc.scalar.activation(warm_c[:], eps_c[:], SQRT, bias=eps_c[:], scale=1.0)

    if stage == "loads":
        nc.vector.tensor_copy(out_sb[:], ha_sb[:])
        nc.sync.dma_start(out=out_d.ap(), in_=out_sb[:])
        return

    # ---- transposes: X = h_A^T (per 128-wide a-tile), [alpha^T; 1] ----
    for i in range(KT):
        tp = trp.tile([128, B_CORE], F32, tag="tr", name=f"trx{i}")
        nc.tensor.transpose(tp[:], ha_sb[:, 128 * i:128 * (i + 1)],
                            ident[:B_CORE, :B_CORE])
        nc.vector.tensor_copy(x_sb[:, B_CORE * i:B_CORE * (i + 1)], tp[:])
    tp = trp.tile([128, B_CORE], F32, tag="tr", name="tral")
    nc.tensor.transpose(tp[:N_EXP + 1], al_sb[:], ident[:B_CORE, :B_CORE])
    nc.vector.tensor_copy(alt_sb[:], tp[:N_EXP + 1])

    # ---- t = h_A @ V^T ; s = t * repeat(alpha, R) ----
    t_ps = [acc.tile([B_CORE, 512], F32, tag=f"t{h}", name=f"t_ps{h}")
            for h in range(NH)]
    dma_engs = (nc.sync, nc.scalar)
    for i in range(KT):
        if i % 2 == 0:   # 1MB chunk = 2 k-tiles, alternating HWDGE queues
            csl = slice(D * i, D * (i + 2))
            dma_engs[(i // 2) % 2].dma_start(
                out=vt_sb[:, csl], in_=vt_d.ap()[:, csl])
        for h in range(NH):
            nc.tensor.matmul(
                t_ps[h][:],
                x_sb[:, B_CORE * i:B_CORE * (i + 1)],
                vt_sb[:, D * i + 512 * h:D * i + 512 * (h + 1)],
                start=(i == 0), stop=(i == KT - 1),
            )
    for h in range(NH):
        o3 = s_sb[:, 512 * h:512 * (h + 1)].rearrange(
            "p (n r) -> p n r", r=R_RANK)
        i3 = t_ps[h][:].rearrange("p (n r) -> p n r", r=R_RANK)
        a3 = al_sb[:, 32 * h:32 * (h + 1)].unsqueeze(-1).broadcast_to(
            [B_CORE, 32, R_RANK])
        nc.vector.tensor_mul(o3, i3, a3)

    if stage == "t":
        nc.sync.dma_start(out=out_d.ap(), in_=s_sb[:])
        return

    # ---- s^T tiles ----
    for j in range(KT):
        tp = trp.tile([128, B_CORE], F32, tag="tr", name=f"trs{j}")
        nc.tensor.transpose(tp[:], s_sb[:, 128 * j:128 * (j + 1)],
                            ident[:B_CORE, :B_CORE])
        nc.vector.tensor_copy(st_sb[:, B_CORE * j:B_CORE * (j + 1)], tp[:])

    # ---- h_T = s @ U^T + h_A @ W^T + [alpha,1] @ [bias_pool; b_base] ----
    h_ps = [acc.tile([B_CORE, 512], F32, tag=f"h{h}", name=f"h_ps{h}")
            for h in range(NH)]
    for j in range(KT):
        if j % 2 == 0:
            csl = slice(D * j, D * (j + 2))
            dma_engs[(j // 2) % 2].dma_start(
                out=ut_sb[:, csl], in_=ut_d.ap()[:, csl])
        for h in range(NH):
            nc.tensor.matmul(
                h_ps[h][:],
                st_sb[:, B_CORE * j:B_CORE * (j + 1)],
                ut_sb[:, D * j + 512 * h:D * j + 512 * (h + 1)],
                start=(j == 0), stop=False,
            )
    for i in range(KT):
        if i % 2 == 0:
            csl = slice(D * i, D * (i + 2))
            dma_engs[(i // 2) % 2].dma_start(
                out=wt_sb[:, csl], in_=wt_d.ap()[:, csl])
        for h in range(NH):
            nc.tensor.matmul(
                h_ps[h][:],
                x_sb[:, B_CORE * i:B_CORE * (i + 1)],
                wt_sb[:, D * i + 512 * h:D * i + 512 * (h + 1)],
                start=False, stop=False,
            )
    for h in range(NH):
        sl = slice(512 * h, 512 * (h + 1))
        nc.tensor.matmul(h_ps[h][:], alt_sb[:], bp_sb[:, sl],
                         start=False, stop=True)
        # h_pre = gamma * h_T + h_A, with row-sums for the mean
        nc.vector.scalar_tensor_tensor(
            out=hpre_sb[:, sl], in0=h_ps[h][:], scalar=gmc_sb[:],
            in1=ha_sb[:, sl], op0=MULT, op1=ADD,
            accum_out=sum_h[h][:])
        # row-sums of squares for the variance (overlaps the other half)
        nc.vector.scalar_tensor_tensor(
            out=sq_sb[:, sl], in0=hpre_sb[:, sl], scalar=1.0,
            in1=hpre_sb[:, sl], op0=MULT, op1=MULT,
            accum_out=ssq_h[h][:])

    if stage == "h":
        nc.sync.dma_start(out=out_d.ap(), in_=hpre_sb[:])
        return

    # ---- LayerNorm via E[x^2] - E[x]^2 ----
    nc.vector.tensor_add(sum_c[:], sum_h[0][:], sum_h[1][:])
    nc.vector.tensor_add(ssq_c[:], ssq_h[0][:], ssq_h[1][:])
    nc.scalar.mul(m_c[:], sum_c[:], 1.0 / D)
    nc.vector.tensor_mul(msq_c[:], m_c[:], m_c[:])
    nc.vector.scalar_tensor_tensor(
        out=var_c[:], in0=ssq_c[:], scalar=1.0 / D, in1=msq_c[:],
        op0=MULT, op1=mybir.AluOpType.subtract)
    nc.scalar.activation(std_c[:], var_c[:], SQRT, bias=eps_c[:], scale=1.0)
    nc.vector.reciprocal(istd_c[:], std_c[:])
    # out = hpre*istd*lns + (lnb - m*istd*lns)
    nc.vector.tensor_mul(nmi_c[:], m_c[:], istd_c[:])
    nc.scalar.mul(nmi_c[:], nmi_c[:], -1.0)
    nc.vector.scalar_tensor_tensor(
        out=t2_sb[:], in0=lnsr_sb[:], scalar=nmi_c[:], in1=lnbr_sb[:],
        op0=MULT, op1=ADD)
    nc.vector.scalar_tensor_tensor(
        out=y_sb[:], in0=hpre_sb[:], scalar=istd_c[:], in1=lnsr_sb[:],
        op0=MULT, op1=MULT)
    nc.vector.tensor_add(out_sb[:], y_sb[:], t2_sb[:])

    nc.sync.dma_start(out=out_d.ap(), in_=out_sb[:])


def _to_sbuf_layout(a):
    """[KT*128, D] logical -> [128, KT*D] partition-major."""
    return np.ascontiguousarray(
        a.reshape(KT, 128, D).transpose(1, 0, 2).reshape(128, KT * D))


def _prep_in_maps(inputs):
    def f32c(x):
        return np.ascontiguousarray(np.asarray(x, dtype=np.float32))

    h_a = f32c(inputs["h_A"])
    alpha = f32c(inputs["alpha"])
    pool = np.asarray(inputs["pool_vectors"], dtype=np.float32)
    w_base = np.asarray(inputs["W_base"], dtype=np.float32)

    # pool_vectors rows: [U_n (D*R) | V_n (R*D) | bias_n (D)]
    u = pool[:, :D * R_RANK].reshape(N_EXP, D, R_RANK)
    v = pool[:, D * R_RANK:2 * D * R_RANK].reshape(N_EXP, R_RANK, D)
    bias_pool = pool[:, 2 * D * R_RANK:]                    # [64, D]
    bb = np.asarray(inputs["b_base"], dtype=np.float32).reshape(1, D)
    bp = f32c(np.concatenate([bias_pool, bb], axis=0))      # [65, D]
    ut = _to_sbuf_layout(
        f32c(u.transpose(0, 2, 1).reshape(N_EXP * R_RANK, D)))  # [(n,r), c]
    vt = _to_sbuf_layout(f32c(v.reshape(N_EXP * R_RANK, D).T))  # [a, (n,r)]
    wt = _to_sbuf_layout(f32c(w_base.T))                        # [a, c]
    lns = f32c(inputs["ln_scale"]).reshape(1, D)
    lnb = f32c(inputs["ln_bias"]).reshape(1, D)
    gm = f32c(inputs["gamma"]).reshape(1, 1)

    in_maps = []
    for k in range(N_CORES):
        rows = slice(B_CORE * k, B_CORE * (k + 1))
        in_maps.append({
            "ha": f32c(h_a[rows]), "al": f32c(alpha[rows]),
            "vt": vt, "ut": ut, "wt": wt, "bp": bp,
            "lns": lns, "lnb": lnb, "gm": gm,
        })
    return in_maps


def get_compiled(mode=None, stage=None):
    key = (mode or MATMUL_MODE, stage or STAGE)
    if key not in _COMPILED:
        _COMPILED[key] = _build(*key)
    return _COMPILED[key]


def kernel(**inputs):
    nc = get_compiled()
    in_maps = _prep_in_maps(inputs)
    res = bass_utils.run_bass_kernel_spmd(
        nc, in_maps, core_ids=list(range(N_CORES)))
    return np.concatenate([r["out"] for r in res.results], axis=0)


# revision 18
# speedup vs baseline: 1.8087x; 1.0247x over previous
"""Trainium2 Bass kernel for the DWA middle layer (moe_routing).

Math (factored form of the reference):
    t     = h_A @ V_flat^T                      # [B, N*R]
    s     = t * repeat(alpha, R, axis=1)        # [B, N*R]
    h_T   = s @ U_flat^T + h_A @ W_base^T + [alpha, 1] @ [bias_pool; b_base]
    out   = LayerNorm(h_A + gamma * h_T) * ln_scale + ln_bias

Sharding: data-parallel over the batch dim (32 rows per core, 8 cores).
Weight matrices are replicated; on the host we only re-lay them out
(transpose/reshape/concat into the SBUF-native partition-major layout)
so the contraction dim lands on SBUF partitions — all arithmetic runs
on device.

All PE matmuls keep the (small) activations stationary and stream the
weight matrices as the moving operand at N=512.  Weight DMAs are issued
in 512KB k-tile chunks interleaved with the matmuls that consume them,
so the PE pipeline runs under the (HBM-bound) weight stream.
"""

import os
from contextlib import ExitStack

import numpy as np

import concourse.bacc as bacc
import concourse.mybir as mybir
import concourse.tile as tile
from concourse import bass_utils, masks

F32 = mybir.dt.float32
F32R = mybir.dt.float32r

D = 1024          # d_A == d_B
B_CORE = 32       # batch rows per core
N_EXP = 64        # experts
R_RANK = 16       # rank per expert
N_CORES = 8
KT = D // 128     # 8 contraction tiles of 128
NH = D // 512     # 2 moving halves of 512

# "f32r" = raw-fp32 single-pass PE mode (faster, slightly relaxed
# multiply precision); "f32" = full two-pass fp32.
MATMUL_MODE = os.environ.get("DWA_MATMUL_MODE", "f32r")
STAGE = os.environ.get("DWA_STAGE", "full")

_COMPILED = {}


def _build(mode, stage="full"):
    nc = bacc.Bacc("TRN2", debug=False, num_devices=N_CORES)
    WDT = F32R if mode == "f32r" else F32

    ha_d = nc.dram_tensor("ha", [B_CORE, D], F32, kind="ExternalInput")
    al_d = nc.dram_tensor("al", [B_CORE, N_EXP], F32, kind="ExternalInput")
    # weights in SBUF-native partition-major layout [128, KT*1024]
    vt_d = nc.dram_tensor("vt", [128, KT * D], WDT, kind="ExternalInput")
    ut_d = nc.dram_tensor("ut", [128, KT * D], WDT, kind="ExternalInput")
    wt_d = nc.dram_tensor("wt", [128, KT * D], WDT, kind="ExternalInput")
    bp_d = nc.dram_tensor("bp", [N_EXP + 1, D], WDT, kind="ExternalInput")
    lns_d = nc.dram_tensor("lns", [1, D], F32, kind="ExternalInput")
    lnb_d = nc.dram_tensor("lnb", [1, D], F32, kind="ExternalInput")
    gm_d = nc.dram_tensor("gm", [1, 1], F32, kind="ExternalInput")
    out_d = nc.dram_tensor("out", [B_CORE, D], F32, kind="ExternalOutput")

    with ExitStack() as ctx:
        tc = ctx.enter_context(tile.TileContext(nc))
        _emit(ctx, tc, WDT, stage, ha_d, al_d, vt_d, ut_d, wt_d, bp_d,
              lns_d, lnb_d, gm_d, out_d)

    nc.compile()
    return nc


def _emit(ctx, tc, WDT, stage, ha_d, al_d, vt_d, ut_d, wt_d, bp_d,
          lns_d, lnb_d, gm_d, out_d):
    nc = tc.nc
    MULT = mybir.AluOpType.mult
    ADD = mybir.AluOpType.add
    SQRT = mybir.ActivationFunctionType.Sqrt

    wpool = ctx.enter_context(tc.tile_pool(name="weights", bufs=1))
    sm = ctx.enter_context(tc.tile_pool(name="small", bufs=1))
    trp = ctx.enter_context(tc.tile_pool(name="trps", bufs=2, space="PSUM"))
    acc = ctx.enter_context(tc.tile_pool(name="acc", bufs=1, space="PSUM"))

    vt_sb = wpool.tile([128, KT * D], WDT, tag="vt")
    ut_sb = wpool.tile([128, KT * D], WDT, tag="ut")
    wt_sb = wpool.tile([128, KT * D], WDT, tag="wt")

    ha_sb = sm.tile([B_CORE, D], F32, tag="ha")
    al_sb = sm.tile([B_CORE, N_EXP + 1], F32, tag="al")  # [alpha | 1]
    bp_sb = sm.tile([N_EXP + 1, D], WDT, tag="bp")
    ident = sm.tile([128, 128], F32, tag="ident")
    x_sb = sm.tile([128, KT * B_CORE], WDT, tag="x")      # h_A^T tiles
    alt_sb = sm.tile([N_EXP + 1, B_CORE], WDT, tag="alt")  # [alpha^T; 1]
    s_sb = sm.tile([B_CORE, D], F32, tag="s")
    st_sb = sm.tile([128, KT * B_CORE], WDT, tag="st")    # s^T tiles
    hpre_sb = sm.tile([B_CORE, D], F32, tag="hpre")
    sq_sb = sm.tile([B_CORE, D], F32, tag="sq")
    y_sb = sm.tile([B_CORE, D], F32, tag="y")
    t2_sb = sm.tile([B_CORE, D], F32, tag="t2")
    out_sb = sm.tile([B_CORE, D], F32, tag="out")
    lnsr_sb = sm.tile([B_CORE, D], F32, tag="lnsr")
    lnbr_sb = sm.tile([B_CORE, D], F32, tag="lnbr")
    gmc_sb = sm.tile([B_CORE, 1], F32, tag="gmc")
    sum_h = [sm.tile([B_CORE, 1], F32, tag=f"sumh{h}", name=f"sumh{h}")
             for h in range(NH)]
    ssq_h = [sm.tile([B_CORE, 1], F32, tag=f"ssqh{h}", name=f"ssqh{h}")
             for h in range(NH)]
    sum_c = sm.tile([B_CORE, 1], F32, tag="sumc")
    m_c = sm.tile([B_CORE, 1], F32, tag="mc")
    msq_c = sm.tile([B_CORE, 1], F32, tag="msqc")
    ssq_c = sm.tile([B_CORE, 1], F32, tag="ssqc")
    var_c = sm.tile([B_CORE, 1], F32, tag="varc")
    std_c = sm.tile([B_CORE, 1], F32, tag="stdc")
    istd_c = sm.tile([B_CORE, 1], F32, tag="istdc")
    nmi_c = sm.tile([B_CORE, 1], F32, tag="nmic")
    eps_c = sm.tile([B_CORE, 1], F32, tag="epsc")
    warm_c = sm.tile([B_CORE, 1], F32, tag="warmc")

    # ---- activation loads first, then the weight chunk stream ----
    nc.sync.dma_start(out=ha_sb[:], in_=ha_d.ap())
    nc.sync.dma_start(out=al_sb[:, :N_EXP], in_=al_d.ap())
    dma_engs = (nc.sync, nc.scalar)
    q = 0
    for w_sb, w_d in ((vt_sb, vt_d), (ut_sb, ut_d), (wt_sb, wt_d)):
        for i in range(0, KT, 2):       # 1MB chunks, alternating HWDGE queues
            csl = slice(D * i, D * (i + 2))
            dma_engs[q % 2].dma_start(out=w_sb[:, csl], in_=w_d.ap()[:, csl])
            q += 1
    # small/late operands via the SWDGE ring (parallel to HWDGE)
    nc.gpsimd.dma_start(out=gmc_sb[:], in_=gm_d.ap().broadcast_to([B_CORE, 1]))
    nc.gpsimd.dma_start(out=bp_sb[:], in_=bp_d.ap())
    nc.gpsimd.dma_start(out=lnsr_sb[:], in_=lns_d.ap().broadcast_to([B_CORE, D]))
    nc.gpsimd.dma_start(out=lnbr_sb[:], in_=lnb_d.ap().broadcast_to([B_CORE, D]))

    nc.vector.memset(al_sb[:, N_EXP:N_EXP + 1], 1.0)
    nc.vector.memset(eps_c[:], 1e-5)
    masks.make_identity(nc, ident[:])
    # preload the Sqrt ACT table off the critical path
    nc.scalar.activation(warm_c[:], eps_c[:], SQRT, bias=eps_c[:], scale=1.0)

    if stage == "loads":
        nc.vector.tensor_copy(out_sb[:], ha_sb[:])
        nc.sync.dma_start(out=out_d.ap(), in_=out_sb[:])
        return

    # ---- transposes: X = h_A^T (per 128-wide a-tile), [alpha^T; 1] ----
    for i in range(KT):
        tp = trp.tile([128, B_CORE], F32, tag="tr", name=f"trx{i}")
        nc.tensor.transpose(tp[:], ha_sb[:, 128 * i:128 * (i + 1)],
                            ident[:B_CORE, :B_CORE])
        nc.vector.tensor_copy(x_sb[:, B_CORE * i:B_CORE * (i + 1)], tp[:])
    tp = trp.tile([128, B_CORE], F32, tag="tr", name="tral")
    nc.tensor.transpose(tp[:N_EXP + 1], al_sb[:], ident[:B_CORE, :B_CORE])
    nc.vector.tensor_copy(alt_sb[:], tp[:N_EXP + 1])

    # ---- t = h_A @ V^T ; s = t * repeat(alpha, R) ----
    t_ps = [acc.tile([B_CORE, 512], F32, tag=f"t{h}", name=f"t_ps{h}")
            for h in range(NH)]
    for i in range(KT):
        for h in range(NH):
            nc.tensor.matmul(
                t_ps[h][:],
                x_sb[:, B_CORE * i:B_CORE * (i + 1)],
                vt_sb[:, D * i + 512 * h:D * i + 512 * (h + 1)],
                start=(i == 0), stop=(i == KT - 1),
            )
    for h in range(NH):
        o3 = s_sb[:, 512 * h:512 * (h + 1)].rearrange(
            "p (n r) -> p n r", r=R_RANK)
        i3 = t_ps[h][:].rearrange("p (n r) -> p n r", r=R_RANK)
        a3 = al_sb[:, 32 * h:32 * (h + 1)].unsqueeze(-1).broadcast_to(
            [B_CORE, 32, R_RANK])
        nc.vector.tensor_mul(o3, i3, a3)

    if stage == "t":
        nc.sync.dma_start(out=out_d.ap(), in_=s_sb[:])
        return

    # ---- s^T tiles ----
    for j in range(KT):
        tp = trp.tile([128, B_CORE], F32, tag="tr", name=f"trs{j}")
        nc.tensor.transpose(tp[:], s_sb[:, 128 * j:128 * (j + 1)],
                            ident[:B_CORE, :B_CORE])
        nc.vector.tensor_copy(st_sb[:, B_CORE * j:B_CORE * (j + 1)], tp[:])

    # ---- h_T = s @ U^T + h_A @ W^T + [alpha,1] @ [bias_pool; b_base] ----
    h_ps = [acc.tile([B_CORE, 512], F32, tag=f"h{h}", name=f"h_ps{h}")
            for h in range(NH)]
    for j in range(KT):
        for h in range(NH):
            nc.tensor.matmul(
                h_ps[h][:],
                st_sb[:, B_CORE * j:B_CORE * (j + 1)],
                ut_sb[:, D * j + 512 * h:D * j + 512 * (h + 1)],
                start=(j == 0), stop=False,
            )
    for i in range(KT):
        for h in range(NH):
            nc.tensor.matmul(
                h_ps[h][:],
                x_sb[:, B_CORE * i:B_CORE * (i + 1)],
                wt_sb[:, D * i + 512 * h:D * i + 512 * (h + 1)],
                start=False, stop=False,
            )
    for h in range(NH):
        sl = slice(512 * h, 512 * (h + 1))
        nc.tensor.matmul(h_ps[h][:], alt_sb[:], bp_sb[:, sl],
                         start=False, stop=True)
        # h_pre = gamma * h_T + h_A, with row-sums for the mean
        nc.vector.scalar_tensor_tensor(
            out=hpre_sb[:, sl], in0=h_ps[h][:], scalar=gmc_sb[:],
            in1=ha_sb[:, sl], op0=MULT, op1=ADD,
            accum_out=sum_h[h][:])
        # row-sums of squares for the variance (overlaps the other half)
        nc.vector.scalar_tensor_tensor(
            out=sq_sb[:, sl], in0=hpre_sb[:, sl], scalar=1.0,
            in1=hpre_sb[:, sl], op0=MULT, op1=MULT,
            accum_out=ssq_h[h][:])

    if stage == "h":
        nc.sync.dma_start(out=out_d.ap(), in_=hpre_sb[:])
        return

    # ---- LayerNorm via E[x^2] - E[x]^2 ----
    nc.vector.tensor_add(sum_c[:], sum_h[0][:], sum_h[1][:])
    nc.vector.tensor_add(ssq_c[:], ssq_h[0][:], ssq_h[1][:])
    nc.scalar.mul(m_c[:], sum_c[:], 1.0 / D)
    nc.vector.tensor_mul(msq_c[:], m_c[:], m_c[:])
    nc.vector.scalar_tensor_tensor(
        out=var_c[:], in0=ssq_c[:], scalar=1.0 / D, in1=msq_c[:],
        op0=MULT, op1=mybir.AluOpType.subtract)
    nc.scalar.activation(std_c[:], var_c[:], SQRT, bias=eps_c[:], scale=1.0)
    nc.vector.reciprocal(istd_c[:], std_c[:])
    # out = hpre*istd*lns + (lnb - m*istd*lns)
    nc.vector.tensor_mul(nmi_c[:], m_c[:], istd_c[:])
    nc.scalar.mul(nmi_c[:], nmi_c[:], -1.0)
    nc.vector.scalar_tensor_tensor(
        out=t2_sb[:], in0=lnsr_sb[:], scalar=nmi_c[:], in1=lnbr_sb[:],
        op0=MULT, op1=ADD)
    nc.vector.scalar_tensor_tensor(
        out=y_sb[:], in0=hpre_sb[:], scalar=istd_c[:], in1=lnsr_sb[:],
        op0=MULT, op1=MULT)
    nc.vector.tensor_add(out_sb[:], y_sb[:], t2_sb[:])

    nc.sync.dma_start(out=out_d.ap(), in_=out_sb[:])


def _to_sbuf_layout(a):
    """[KT*128, D] logical -> [128, KT*D] partition-major."""
    return np.ascontiguousarray(
        a.reshape(KT, 128, D).transpose(1, 0, 2).reshape(128, KT * D))


def _prep_in_maps(inputs):
    def f32c(x):
        return np.ascontiguousarray(np.asarray(x, dtype=np.float32))

    h_a = f32c(inputs["h_A"])
    alpha = f32c(inputs["alpha"])
    pool = np.asarray(inputs["pool_vectors"], dtype=np.float32)
    w_base = np.asarray(inputs["W_base"], dtype=np.float32)

    # pool_vectors rows: [U_n (D*R) | V_n (R*D) | bias_n (D)]
    u = pool[:, :D * R_RANK].reshape(N_EXP, D, R_RANK)
    v = pool[:, D * R_RANK:2 * D * R_RANK].reshape(N_EXP, R_RANK, D)
    bias_pool = pool[:, 2 * D * R_RANK:]                    # [64, D]
    bb = np.asarray(inputs["b_base"], dtype=np.float32).reshape(1, D)
    bp = f32c(np.concatenate([bias_pool, bb], axis=0))      # [65, D]
    ut = _to_sbuf_layout(
        f32c(u.transpose(0, 2, 1).reshape(N_EXP * R_RANK, D)))  # [(n,r), c]
    vt = _to_sbuf_layout(f32c(v.reshape(N_EXP * R_RANK, D).T))  # [a, (n,r)]
    wt = _to_sbuf_layout(f32c(w_base.T))                        # [a, c]
    lns = f32c(inputs["ln_scale"]).reshape(1, D)
    lnb = f32c(inputs["ln_bias"]).reshape(1, D)
    gm = f32c(inputs["gamma"]).reshape(1, 1)

    in_maps = []
    for k in range(N_CORES):
        rows = slice(B_CORE * k, B_CORE * (k + 1))
        in_maps.append({
            "ha": f32c(h_a[rows]), "al": f32c(alpha[rows]),
            "vt": vt, "ut": ut, "wt": wt, "bp": bp,
            "lns": lns, "lnb": lnb, "gm": gm,
        })
    return in_maps


def get_compiled(mode=None, stage=None):
    key = (mode or MATMUL_MODE, stage or STAGE)
    if key not in _COMPILED:
        _COMPILED[key] = _build(*key)
    return _COMPILED[key]


def kernel(**inputs):
    nc = get_compiled()
    in_maps = _prep_in_maps(inputs)
    res = bass_utils.run_bass_kernel_spmd(
        nc, in_maps, core_ids=list(range(N_CORES)))
    return np.concatenate([r["out"] for r in res.results], axis=0)


# revision 19
# speedup vs baseline: 1.8738x; 1.0360x over previous
"""Trainium2 Bass kernel for the DWA middle layer (moe_routing).

Math (factored form of the reference):
    t     = h_A @ V_flat^T                      # [B, N*R]
    s     = t * repeat(alpha, R, axis=1)        # [B, N*R]
    h_T   = s @ U_flat^T + h_A @ W_base^T + [alpha, 1] @ [bias_pool; b_base]
    out   = LayerNorm(h_A + gamma * h_T) * ln_scale + ln_bias

Sharding: data-parallel over the batch dim (32 rows per core, 8 cores).
Weight matrices are replicated; on the host we only re-lay them out
(transpose/reshape/concat into the SBUF-native partition-major layout)
so the contraction dim lands on SBUF partitions — all arithmetic runs
on device.

All PE matmuls keep the (small) activations stationary and stream the
weight matrices as the moving operand at N=512.  Weight DMAs are issued
in 512KB k-tile chunks interleaved with the matmuls that consume them,
so the PE pipeline runs under the (HBM-bound) weight stream.
"""

import os
from contextlib import ExitStack

import numpy as np

import concourse.bacc as bacc
import concourse.mybir as mybir
import concourse.tile as tile
from concourse import bass_utils, masks

F32 = mybir.dt.float32
F32R = mybir.dt.float32r

D = 1024          # d_A == d_B
B_CORE = 32       # batch rows per core
N_EXP = 64        # experts
R_RANK = 16       # rank per expert
N_CORES = 8
KT = D // 128     # 8 contraction tiles of 128
NH = D // 512     # 2 moving halves of 512

# "f32r" = raw-fp32 single-pass PE mode (faster, slightly relaxed
# multiply precision); "f32" = full two-pass fp32.
MATMUL_MODE = os.environ.get("DWA_MATMUL_MODE", "f32r")
STAGE = os.environ.get("DWA_STAGE", "full")

_COMPILED = {}


def _build(mode, stage="full"):
    nc = bacc.Bacc("TRN2", debug=False, num_devices=N_CORES,
                   enable_partition_id=False)
    WDT = F32R if mode == "f32r" else F32

    ha_d = nc.dram_tensor("ha", [B_CORE, D], F32, kind="ExternalInput")
    al_d = nc.dram_tensor("al", [B_CORE, N_EXP], F32, kind="ExternalInput")
    # weights in SBUF-native partition-major layout [128, KT*1024]
    vt_d = nc.dram_tensor("vt", [128, KT * D], WDT, kind="ExternalInput")
    ut_d = nc.dram_tensor("ut", [128, KT * D], WDT, kind="ExternalInput")
    wt_d = nc.dram_tensor("wt", [128, KT * D], WDT, kind="ExternalInput")
    bp_d = nc.dram_tensor("bp", [N_EXP + 1, D], WDT, kind="ExternalInput")
    lns_d = nc.dram_tensor("lns", [1, D], F32, kind="ExternalInput")
    lnb_d = nc.dram_tensor("lnb", [1, D], F32, kind="ExternalInput")
    gm_d = nc.dram_tensor("gm", [1, 1], F32, kind="ExternalInput")
    out_d = nc.dram_tensor("out", [B_CORE, D], F32, kind="ExternalOutput")

    with ExitStack() as ctx:
        tc = ctx.enter_context(tile.TileContext(nc))
        _emit(ctx, tc, WDT, stage, ha_d, al_d, vt_d, ut_d, wt_d, bp_d,
              lns_d, lnb_d, gm_d, out_d)

    nc.compile()
    return nc


def _emit(ctx, tc, WDT, stage, ha_d, al_d, vt_d, ut_d, wt_d, bp_d,
          lns_d, lnb_d, gm_d, out_d):
    nc = tc.nc
    MULT = mybir.AluOpType.mult
    ADD = mybir.AluOpType.add
    SQRT = mybir.ActivationFunctionType.Sqrt

    wpool = ctx.enter_context(tc.tile_pool(name="weights", bufs=1))
    sm = ctx.enter_context(tc.tile_pool(name="small", bufs=1))
    trp = ctx.enter_context(tc.tile_pool(name="trps", bufs=2, space="PSUM"))
    acc = ctx.enter_context(tc.tile_pool(name="acc", bufs=1, space="PSUM"))

    vt_sb = wpool.tile([128, KT * D], WDT, tag="vt")
    ut_sb = wpool.tile([128, KT * D], WDT, tag="ut")
    wt_sb = wpool.tile([128, KT * D], WDT, tag="wt")

    ha_sb = sm.tile([B_CORE, D], F32, tag="ha")
    al_sb = sm.tile([B_CORE, N_EXP + 1], F32, tag="al")  # [alpha | 1]
    bp_sb = sm.tile([N_EXP + 1, D], WDT, tag="bp")
    ident = sm.tile([128, 128], F32, tag="ident")
    x_sb = sm.tile([128, KT * B_CORE], WDT, tag="x")      # h_A^T tiles
    alt_sb = sm.tile([N_EXP + 1, B_CORE], WDT, tag="alt")  # [alpha^T; 1]
    s_sb = sm.tile([B_CORE, D], F32, tag="s")
    st_sb = sm.tile([128, KT * B_CORE], WDT, tag="st")    # s^T tiles
    hpre_sb = sm.tile([B_CORE, D], F32, tag="hpre")
    sq_sb = sm.tile([B_CORE, D], F32, tag="sq")
    y_sb = sm.tile([B_CORE, D], F32, tag="y")
    t2_sb = sm.tile([B_CORE, D], F32, tag="t2")
    out_sb = sm.tile([B_CORE, D], F32, tag="out")
    lnsr_sb = sm.tile([B_CORE, D], F32, tag="lnsr")
    lnbr_sb = sm.tile([B_CORE, D], F32, tag="lnbr")
    gmc_sb = sm.tile([B_CORE, 1], F32, tag="gmc")
    sum_h = [sm.tile([B_CORE, 1], F32, tag=f"sumh{h}", name=f"sumh{h}")
             for h in range(NH)]
    ssq_h = [sm.tile([B_CORE, 1], F32, tag=f"ssqh{h}", name=f"ssqh{h}")
             for h in range(NH)]
    sum_c = sm.tile([B_CORE, 1], F32, tag="sumc")
    m_c = sm.tile([B_CORE, 1], F32, tag="mc")
    msq_c = sm.tile([B_CORE, 1], F32, tag="msqc")
    ssq_c = sm.tile([B_CORE, 1], F32, tag="ssqc")
    var_c = sm.tile([B_CORE, 1], F32, tag="varc")
    std_c = sm.tile([B_CORE, 1], F32, tag="stdc")
    istd_c = sm.tile([B_CORE, 1], F32, tag="istdc")
    nmi_c = sm.tile([B_CORE, 1], F32, tag="nmic")
    eps_c = sm.tile([B_CORE, 1], F32, tag="epsc")
    warm_c = sm.tile([B_CORE, 1], F32, tag="warmc")

    # ---- activation loads first, then the weight chunk stream ----
    nc.sync.dma_start(out=ha_sb[:], in_=ha_d.ap())
    nc.sync.dma_start(out=al_sb[:, :N_EXP], in_=al_d.ap())
    dma_engs = (nc.sync, nc.scalar)
    q = 0
    for w_sb, w_d in ((vt_sb, vt_d), (ut_sb, ut_d), (wt_sb, wt_d)):
        for i in range(0, KT, 2):       # 1MB chunks, alternating HWDGE queues
            csl = slice(D * i, D * (i + 2))
            dma_engs[q % 2].dma_start(out=w_sb[:, csl], in_=w_d.ap()[:, csl])
            q += 1
    # tail-only operands go last on the HWDGE rings (behind the weights);
    # gamma (needed right after the matmuls) rides the parallel SWDGE ring
    nc.gpsimd.dma_start(out=gmc_sb[:], in_=gm_d.ap().broadcast_to([B_CORE, 1]))
    nc.sync.dma_start(out=bp_sb[:], in_=bp_d.ap())
    nc.sync.dma_start(out=lnsr_sb[:], in_=lns_d.ap().broadcast_to([B_CORE, D]))
    nc.scalar.dma_start(out=lnbr_sb[:], in_=lnb_d.ap().broadcast_to([B_CORE, D]))

    nc.vector.memset(al_sb[:, N_EXP:N_EXP + 1], 1.0)
    nc.vector.memset(eps_c[:], 1e-5)
    masks.make_identity(nc, ident[:])
    # preload both ACT tables (Square, Sqrt) off the critical path
    nc.scalar.activation(warm_c[:], eps_c[:],
                         mybir.ActivationFunctionType.Square)
    nc.scalar.activation(warm_c[:], eps_c[:], SQRT, bias=eps_c[:], scale=1.0)

    if stage == "loads":
        nc.vector.tensor_copy(out_sb[:], ha_sb[:])
        nc.sync.dma_start(out=out_d.ap(), in_=out_sb[:])
        return

    # ---- transposes: X = h_A^T (per 128-wide a-tile), [alpha^T; 1] ----
    for i in range(KT):
        tp = trp.tile([128, B_CORE], F32, tag="tr", name=f"trx{i}")
        nc.tensor.transpose(tp[:], ha_sb[:, 128 * i:128 * (i + 1)],
                            ident[:B_CORE, :B_CORE])
        nc.vector.tensor_copy(x_sb[:, B_CORE * i:B_CORE * (i + 1)], tp[:])
    tp = trp.tile([128, B_CORE], F32, tag="tr", name="tral")
    nc.tensor.transpose(tp[:N_EXP + 1], al_sb[:], ident[:B_CORE, :B_CORE])
    nc.vector.tensor_copy(alt_sb[:], tp[:N_EXP + 1])

    # ---- t = h_A @ V^T ; s = t * repeat(alpha, R) ----
    t_ps = [acc.tile([B_CORE, 512], F32, tag=f"t{h}", name=f"t_ps{h}")
            for h in range(NH)]
    for i in range(KT):
        for h in range(NH):
            nc.tensor.matmul(
                t_ps[h][:],
                x_sb[:, B_CORE * i:B_CORE * (i + 1)],
                vt_sb[:, D * i + 512 * h:D * i + 512 * (h + 1)],
                start=(i == 0), stop=(i == KT - 1),
            )
    for h in range(NH):
        o3 = s_sb[:, 512 * h:512 * (h + 1)].rearrange(
            "p (n r) -> p n r", r=R_RANK)
        i3 = t_ps[h][:].rearrange("p (n r) -> p n r", r=R_RANK)
        a3 = al_sb[:, 32 * h:32 * (h + 1)].unsqueeze(-1).broadcast_to(
            [B_CORE, 32, R_RANK])
        nc.vector.tensor_mul(o3, i3, a3)

    if stage == "t":
        nc.sync.dma_start(out=out_d.ap(), in_=s_sb[:])
        return

    # ---- s^T tiles ----
    for j in range(KT):
        tp = trp.tile([128, B_CORE], F32, tag="tr", name=f"trs{j}")
        nc.tensor.transpose(tp[:], s_sb[:, 128 * j:128 * (j + 1)],
                            ident[:B_CORE, :B_CORE])
        nc.vector.tensor_copy(st_sb[:, B_CORE * j:B_CORE * (j + 1)], tp[:])

    # ---- h_T = s @ U^T + h_A @ W^T + [alpha,1] @ [bias_pool; b_base] ----
    h_ps = [acc.tile([B_CORE, 512], F32, tag=f"h{h}", name=f"h_ps{h}")
            for h in range(NH)]
    for j in range(KT):
        for h in range(NH):
            nc.tensor.matmul(
                h_ps[h][:],
                st_sb[:, B_CORE * j:B_CORE * (j + 1)],
                ut_sb[:, D * j + 512 * h:D * j + 512 * (h + 1)],
                start=(j == 0), stop=False,
            )
    for i in range(KT):
        for h in range(NH):
            nc.tensor.matmul(
                h_ps[h][:],
                x_sb[:, B_CORE * i:B_CORE * (i + 1)],
                wt_sb[:, D * i + 512 * h:D * i + 512 * (h + 1)],
                start=False, stop=False,
            )
    for h in range(NH):
        sl = slice(512 * h, 512 * (h + 1))
        nc.tensor.matmul(h_ps[h][:], alt_sb[:], bp_sb[:, sl],
                         start=False, stop=True)
        # h_pre = gamma * h_T + h_A, with row-sums for the mean
        nc.vector.scalar_tensor_tensor(
            out=hpre_sb[:, sl], in0=h_ps[h][:], scalar=gmc_sb[:],
            in1=ha_sb[:, sl], op0=MULT, op1=ADD,
            accum_out=sum_h[h][:])
        # row-sums of squares on the Scalar engine (parallel to DVE)
        nc.scalar.activation(sq_sb[:, sl], hpre_sb[:, sl],
                             mybir.ActivationFunctionType.Square,
                             accum_out=ssq_h[h][:])

    if stage == "h":
        nc.sync.dma_start(out=out_d.ap(), in_=hpre_sb[:])
        return

    # ---- LayerNorm via E[x^2] - E[x]^2 ----
    nc.vector.tensor_add(sum_c[:], sum_h[0][:], sum_h[1][:])
    nc.vector.tensor_add(ssq_c[:], ssq_h[0][:], ssq_h[1][:])
    nc.scalar.mul(m_c[:], sum_c[:], 1.0 / D)
    nc.vector.tensor_mul(msq_c[:], m_c[:], m_c[:])
    nc.vector.scalar_tensor_tensor(
        out=var_c[:], in0=ssq_c[:], scalar=1.0 / D, in1=msq_c[:],
        op0=MULT, op1=mybir.AluOpType.subtract)
    nc.scalar.activation(std_c[:], var_c[:], SQRT, bias=eps_c[:], scale=1.0)
    nc.vector.reciprocal(istd_c[:], std_c[:])
    # out = hpre*istd*lns + (lnb - m*istd*lns), in halves overlapped
    # with the output DMA
    nc.vector.tensor_mul(nmi_c[:], m_c[:], istd_c[:])
    nc.scalar.mul(nmi_c[:], nmi_c[:], -1.0)
    for h in range(NH):
        sl = slice(512 * h, 512 * (h + 1))
        nc.vector.scalar_tensor_tensor(
            out=t2_sb[:, sl], in0=lnsr_sb[:, sl], scalar=nmi_c[:],
            in1=lnbr_sb[:, sl], op0=MULT, op1=ADD)
        nc.vector.scalar_tensor_tensor(
            out=y_sb[:, sl], in0=hpre_sb[:, sl], scalar=istd_c[:],
            in1=lnsr_sb[:, sl], op0=MULT, op1=MULT)
        nc.vector.tensor_add(out_sb[:, sl], y_sb[:, sl], t2_sb[:, sl])
        nc.sync.dma_start(out=out_d.ap()[:, sl], in_=out_sb[:, sl])


def _to_sbuf_layout(a):
    """[KT*128, D] logical -> [128, KT*D] partition-major."""
    return np.ascontiguousarray(
        a.reshape(KT, 128, D).transpose(1, 0, 2).reshape(128, KT * D))


def _prep_in_maps(inputs):
    def f32c(x):
        return np.ascontiguousarray(np.asarray(x, dtype=np.float32))

    h_a = f32c(inputs["h_A"])
    alpha = f32c(inputs["alpha"])
    pool = np.asarray(inputs["pool_vectors"], dtype=np.float32)
    w_base = np.asarray(inputs["W_base"], dtype=np.float32)

    # pool_vectors rows: [U_n (D*R) | V_n (R*D) | bias_n (D)]
    u = pool[:, :D * R_RANK].reshape(N_EXP, D, R_RANK)
    v = pool[:, D * R_RANK:2 * D * R_RANK].reshape(N_EXP, R_RANK, D)
    bias_pool = pool[:, 2 * D * R_RANK:]                    # [64, D]
    bb = np.asarray(inputs["b_base"], dtype=np.float32).reshape(1, D)
    bp = f32c(np.concatenate([bias_pool, bb], axis=0))      # [65, D]
    ut = _to_sbuf_layout(
        f32c(u.transpose(0, 2, 1).reshape(N_EXP * R_RANK, D)))  # [(n,r), c]
    vt = _to_sbuf_layout(f32c(v.reshape(N_EXP * R_RANK, D).T))  # [a, (n,r)]
    wt = _to_sbuf_layout(f32c(w_base.T))                        # [a, c]
    lns = f32c(inputs["ln_scale"]).reshape(1, D)
    lnb = f32c(inputs["ln_bias"]).reshape(1, D)
    gm = f32c(inputs["gamma"]).reshape(1, 1)

    in_maps = []
    for k in range(N_CORES):
        rows = slice(B_CORE * k, B_CORE * (k + 1))
        in_maps.append({
            "ha": f32c(h_a[rows]), "al": f32c(alpha[rows]),
            "vt": vt, "ut": ut, "wt": wt, "bp": bp,
            "lns": lns, "lnb": lnb, "gm": gm,
        })
    return in_maps


def get_compiled(mode=None, stage=None):
    key = (mode or MATMUL_MODE, stage or STAGE)
    if key not in _COMPILED:
        _COMPILED[key] = _build(*key)
    return _COMPILED[key]


def kernel(**inputs):
    nc = get_compiled()
    in_maps = _prep_in_maps(inputs)
    res = bass_utils.run_bass_kernel_spmd(
        nc, in_maps, core_ids=list(range(N_CORES)))
    return np.concatenate([r["out"] for r in res.results], axis=0)
